# revision 7
# baseline (speedup 1.0000x reference)
"""CTC loss (focal-reweighted) Trainium2 Bass kernel — transposed-DP design.

Strategy: pure data parallel over batch (128 examples -> 8 cores x 16).
Per core:
  - stream x tiles of [8 examples x 16 timesteps, C] (host-permuted rows so
    each tile is one contiguous DMA); ACT exp with accum_out -> softmax
    denominators Z[b,t] (in-place, after the gather has read the raw tile)
  - ap_gather (GPSIMD) pulls each (b,t) row's 51 extended-label-state raw
    logits out of the tile; PE transposes the gather [128,64] -> [64,128]
    (states on partitions); a PE "duplication" matmul builds the [103,256]
    per-chunk layout (row 0 spare, rows 1-51 alpha-emissions, rows 52-102 a
    second copy for the skip term); ACT exps it; one DVE multiply applies
    the per-example skip mask.
  - CTC forward DP in rescaled prob space, states-on-partitions: each step
    is ONE PE matmul with a constant [103,103] transition matrix (bands +
    skip coupling + a column-sum row for renorm) and ONE DVE multiply by
    the emission chunk. Renorm every 8 steps via the free column-sum row
    (reciprocal + rank-1 PE broadcast + one extra multiply).
  - readout entirely in free-dim-per-example [1,16] layout: selection mask
    + PE column-sum for v, exponent/mantissa-split log, sum log S from the
    logged renorm factors, sum log Z via PE partition reduction. No DRAM
    bounce, no SBUF->SBUF reshuffle DMAs.
Host: shards inputs, builds index/mask/transition constants, means the 128
per-example losses.
"""

import numpy as np

import concourse.bass as bass
import concourse.bacc as bacc
import concourse.tile as tile
from concourse import mybir
from concourse import bass_utils

B, T, C, L = 128, 160, 6625, 25
NCORES = 8
BL = B // NCORES          # 16 examples per core
S = 2 * L + 1             # 51 extended states
NI = 64                   # ap_gather num_idxs (S padded to a multiple of 16)
TBJ = 10                  # t-blocks of 16 timesteps
NT = 2 * TBJ              # 20 streaming tiles of [128, C]
RENORM = 8
NREN = 19                 # renorms at t = 8, 16, ..., 152
SP = 2 * S + 1            # 103: DP partition rows (1 sum row + 51 + 51)

F32 = mybir.dt.float32
I16 = mybir.dt.int16
U32 = mybir.dt.uint32
LN2 = 0.6931471805599453


def _build_kernel():
    nc = bacc.Bacc("TRN2", target_bir_lowering=False, debug=False)
    x = nc.dram_tensor("x", [BL * T, C], F32, kind="ExternalInput").ap()
    gidx = nc.dram_tensor("gidx", [128, NT * 4], I16, kind="ExternalInput").ap()
    mc = nc.dram_tensor("mc", [SP, SP], F32, kind="ExternalInput").ap()
    dup = nc.dram_tensor("dup", [NI, SP], F32, kind="ExternalInput").ap()
    iden = nc.dram_tensor("iden", [128, 128], F32, kind="ExternalInput").ap()
    skipm = nc.dram_tensor("skipm", [SP, 16 * BL], F32, kind="ExternalInput").ap()
    selt = nc.dram_tensor("selt", [S + 1, BL], F32, kind="ExternalInput").ap()
    bsel = nc.dram_tensor("bsel", [128, 2 * BL], F32, kind="ExternalInput").ap()
    ones1 = nc.dram_tensor("ones1", [1, SP], F32, kind="ExternalInput").ap()
    ones52 = nc.dram_tensor("ones52", [S + 1, 1], F32, kind="ExternalInput").ap()
    initm = nc.dram_tensor("initm", [SP, BL], F32, kind="ExternalInput").ap()
    loss16 = nc.dram_tensor("loss16", [1, BL], F32, kind="ExternalOutput").ap()

    with tile.TileContext(nc) as tc:
        with (
            tc.tile_pool(name="xio", bufs=5) as xio,
            tc.tile_pool(name="small", bufs=1) as small,
            tc.tile_pool(name="ga", bufs=2) as gap,
            tc.tile_pool(name="rawt", bufs=2) as rawp,
            tc.tile_pool(name="pstep", bufs=2, space="PSUM") as pstep,
            tc.tile_pool(name="ptr", bufs=2, space="PSUM") as ptr,
            tc.tile_pool(name="pdup", bufs=2, space="PSUM") as pdup,
        ):
            xv = x.rearrange("(n p) c -> n p c", p=128)

            # ---- first two x tiles go out before anything else ----
            xts = [xio.tile([128, C], F32, name="xt") for i in range(2)]
            nc.sync.dma_start(out=xts[0][:, :], in_=xv[0, :, :])
            nc.sync.dma_start(out=xts[1][:, :], in_=xv[1, :, :])

            # ---- constants (scalar HWDGE ring keeps the sync ring free) ----
            gidx_sb = small.tile([128, NT * 4], I16)
            nc.scalar.dma_start(out=gidx_sb[:, :], in_=gidx[:, :])
            mc_sb = small.tile([SP, SP], F32)
            nc.scalar.dma_start(out=mc_sb[:, :], in_=mc[:, :])
            dup_sb = small.tile([NI, SP], F32)
            nc.scalar.dma_start(out=dup_sb[:, :], in_=dup[:, :])
            iden_sb = small.tile([128, 128], F32)
            nc.scalar.dma_start(out=iden_sb[:, :], in_=iden[:, :])
            skipm_sb = small.tile([SP, 16 * BL], F32)
            nc.scalar.dma_start(out=skipm_sb[:, :], in_=skipm[:, :])
            selt_sb = small.tile([S + 1, BL], F32)
            nc.scalar.dma_start(out=selt_sb[:, :], in_=selt[:, :])
            bsel_sb = small.tile([128, 2 * BL], F32)
            nc.scalar.dma_start(out=bsel_sb[:, :], in_=bsel[:, :])
            ones1_sb = small.tile([1, SP], F32)
            nc.scalar.dma_start(out=ones1_sb[:, :], in_=ones1[:, :])
            ones52_sb = small.tile([S + 1, 1], F32)
            nc.scalar.dma_start(out=ones52_sb[:, :], in_=ones52[:, :])
            initm_sb = small.tile([SP, BL], F32)
            nc.scalar.dma_start(out=initm_sb[:, :], in_=initm[:, :])

            Z = small.tile([128, NT], F32)
            Cbuf = small.tile([1, BL * NREN], F32)
            A = small.tile([SP, BL], F32)
            Bb = small.tile([SP, BL], F32)
            rec = small.tile([1, BL], F32)
            nc.vector.memset(A[:, :], 0.0)
            nc.vector.memset(Bb[:, :], 0.0)

            ee_tiles = []
            cur, nxt = A, Bb
            kren = 0

            for j in range(TBJ):
                # ---- stream pair j (tiles 2j, 2j+1) + build chunk j ----
                rawt = rawp.tile([NI, 2 * 128], F32)
                rtv = rawt[:, :].rearrange("p (t b) -> p b t", b=16)
                for o in range(2):
                    i = 2 * j + o
                    if i < 2:
                        xt = xts[i]
                    else:
                        xt = xio.tile([128, C], F32)
                        nc.sync.dma_start(out=xt[:, :], in_=xv[i, :, :])
                    ga = gap.tile([128, NI], F32)
                    nc.gpsimd.ap_gather(
                        out_ap=ga[:, :].rearrange("p (n d) -> p n d", d=1),
                        in_ap=xt[:, :].rearrange("p (n d) -> p n d", d=1),
                        idxs_ap=gidx_sb[:, i * 4:(i + 1) * 4],
                        channels=128, num_elems=C, d=1, num_idxs=NI,
                    )
                    pt = ptr.tile([NI, 128], F32)
                    nc.tensor.transpose(pt[:, :], ga[:, :], iden_sb[:, :])
                    # psum -> sbuf with (b-major) -> (t-major) column reorder
                    nc.scalar.copy(
                        out=rtv[:, 8 * o:8 * o + 8, :],
                        in_=pt[:, :].rearrange("p (b t) -> p b t", t=16),
                    )
                    # full-tile exp in place (after the gather) for Z
                    nc.scalar.activation(out=xt[:, :], in_=xt[:, :],
                                         func=mybir.ActivationFunctionType.Exp,
                                         accum_out=Z[:, i:i + 1])
                ee = small.tile([SP, 2 * 128], F32, tag=f"ee{j}")
                pd = pdup.tile([SP, 2 * 128], F32)
                nc.tensor.matmul(pd[:, :], dup_sb[:, :], rawt[:, :],
                                 start=True, stop=True)
                nc.scalar.activation(out=ee[:, :], in_=pd[:, :],
                                     func=mybir.ActivationFunctionType.Exp)
                nc.vector.tensor_mul(out=ee[:, :], in0=ee[:, :],
                                     in1=skipm_sb[:, :])
                ee_tiles.append(ee)

                # ---- DP group j: steps t = 16j .. 16j+15 (t>=1) ----
                if j == 0:
                    # init: alpha_0 rows 1,2 and askip_0 rows 52,53 from ee0[t=0]
                    # (single base-partition-0 op: engines cannot start at p=1)
                    nc.vector.tensor_mul(out=cur[:, :], in0=ee[:, 0:BL],
                                         in1=initm_sb[:, :])
                for t in range(max(1, 16 * j), 16 * j + 16):
                    eet = ee[:, (t % 16) * BL:(t % 16 + 1) * BL]
                    ps = pstep.tile([SP, BL], F32)
                    nc.tensor.matmul(ps[:, :], mc_sb[:, :], cur[:, :],
                                     start=True, stop=True)
                    nc.vector.tensor_mul(out=nxt[:, :], in0=ps[:, :], in1=eet)
                    if t % RENORM == 0 and t <= 152:
                        # renorm by c = colsum(alpha_{t-1}) (psum row 0)
                        nc.vector.tensor_copy(
                            out=Cbuf[:, :].rearrange(
                                "p (b k) -> p b k", k=NREN)[:, :, kren:kren + 1],
                            in_=ps[0:1, :].rearrange("p (b k) -> p b k", k=1),
                        )
                        nc.vector.reciprocal(out=rec[:, :], in_=ps[0:1, :])
                        pb = pstep.tile([SP, BL], F32, name="pb", bufs=1)
                        nc.tensor.matmul(pb[:, :], ones1_sb[:, :], rec[:, :],
                                         start=True, stop=True)
                        nc.vector.tensor_mul(out=nxt[:, :], in0=nxt[:, :],
                                             in1=pb[:, :])
                        kren += 1
                    cur, nxt = nxt, cur
            assert kren == NREN

            # ---- readout ----
            # v = alpha[2*len] + alpha[2*len-1] via selection mask + PE colsum
            vm = small.tile([S + 1, BL], F32)
            nc.vector.tensor_mul(out=vm[:, :], in0=cur[0:S + 1, :],
                                 in1=selt_sb[:, :])
            pv = pstep.tile([SP, BL], F32, name="pb", bufs=1)
            nc.tensor.matmul(pv[0:1, :], ones52_sb[:, :], vm[:, :],
                             start=True, stop=True)
            v = small.tile([1, BL], F32)
            nc.vector.tensor_copy(out=v[:, :], in_=pv[0:1, :])
            # log v with v possibly huge/tiny: IEEE exponent/mantissa split
            # (the ACT Ln table is only accurate for inputs in ~e^[-40, 40])
            ebits = small.tile([1, BL], U32)
            mbits = small.tile([1, BL], U32)
            exf = small.tile([1, BL], F32)
            nc.vector.tensor_scalar(out=ebits[:, :], in0=v[:, :].bitcast(U32),
                                    scalar1=23, scalar2=None,
                                    op0=mybir.AluOpType.logical_shift_right)
            nc.vector.tensor_copy(out=exf[:, :], in_=ebits[:, :])
            nc.vector.tensor_scalar(out=mbits[:, :], in0=v[:, :].bitcast(U32),
                                    scalar1=0x7FFFFF, scalar2=0x3F800000,
                                    op0=mybir.AluOpType.bitwise_and,
                                    op1=mybir.AluOpType.bitwise_or)
            nc.scalar.activation(out=v[:, :], in_=mbits[:, :].bitcast(F32),
                                 func=mybir.ActivationFunctionType.Ln)
            # v = ln(mantissa) + (exponent - 127) * ln2
            nc.vector.tensor_scalar(out=exf[:, :], in0=exf[:, :],
                                    scalar1=LN2, scalar2=-127.0 * LN2,
                                    op0=mybir.AluOpType.mult,
                                    op1=mybir.AluOpType.add)
            nc.vector.tensor_add(out=v[:, :], in0=v[:, :], in1=exf[:, :])
            # slS = sum over renorms of ln c
            nc.scalar.activation(out=Cbuf[:, :], in_=Cbuf[:, :],
                                 func=mybir.ActivationFunctionType.Ln)
            slS = small.tile([1, BL], F32)
            nc.vector.reduce_sum(
                out=slS[:, :],
                in_=Cbuf[:, :].rearrange("p (b k) -> p b k", k=NREN),
                axis=mybir.AxisListType.X)
            # slZ = sum over t of ln Z[b, t]; Z cols i=2j+o, partition b*16+tf
            lnz = small.tile([128, NT], F32)
            nc.scalar.activation(out=lnz[:, :], in_=Z[:, :],
                                 func=mybir.ActivationFunctionType.Ln)
            zred = small.tile([128, 2], F32)
            nc.vector.reduce_sum(
                out=zred[:, :],
                in_=lnz[:, :].rearrange("p (j o) -> p o j", o=2),
                axis=mybir.AxisListType.X)
            pz = pstep.tile([SP, BL], F32, name="pb", bufs=1)
            nc.tensor.matmul(pz[0:1, :], zred[:, 0:1], bsel_sb[:, 0:BL],
                             start=True, stop=False)
            nc.tensor.matmul(pz[0:1, :], zred[:, 1:2], bsel_sb[:, BL:2 * BL],
                             start=False, stop=True)
            slZ = small.tile([1, BL], F32)
            nc.vector.tensor_copy(out=slZ[:, :], in_=pz[0:1, :])
            # loss = slZ - (log v + slS)
            lt = small.tile([1, BL], F32)
            nc.vector.tensor_add(out=lt[:, :], in0=v[:, :], in1=slS[:, :])
            nc.vector.tensor_tensor(out=lt[:, :], in0=slZ[:, :], in1=lt[:, :],
                                    op=mybir.AluOpType.subtract)
            # focal: w = (exp(-loss) - 1)^2 ; out = loss * w
            em = small.tile([1, BL], F32)
            nc.vector.tensor_scalar_min(out=em[:, :], in0=lt[:, :], scalar1=80.0)
            nc.scalar.activation(out=em[:, :], in_=em[:, :],
                                 func=mybir.ActivationFunctionType.Exp,
                                 scale=-1.0)
            nc.vector.tensor_scalar_add(out=em[:, :], in0=em[:, :], scalar1=-1.0)
            nc.vector.tensor_mul(out=em[:, :], in0=em[:, :], in1=em[:, :])
            nc.vector.tensor_mul(out=lt[:, :], in0=lt[:, :], in1=em[:, :])
            nc.scalar.dma_start(out=loss16[:, :], in_=lt[:, :])

    nc.compile()
    return nc


def _shared_consts():
    """Example-independent constants."""
    mc = np.zeros((SP, SP), np.float32)
    mc[1:S + 1, 0] = 1.0                       # colsum row (psum row 0)
    for m in range(1, S + 1):
        sp = m - 1
        mc[m, m] = 1.0                          # alpha[s']
        if sp >= 1:
            mc[m - 1, m] = 1.0                  # alpha[s'-1]
        if sp >= 2:
            mc[m + 49, m] = 1.0                 # askip[s'-2] (row 52+s'-2)
    mc[:, S + 1:SP] = mc[:, 1:S + 1]            # duplicated output block
    dup = np.zeros((NI, SP), np.float32)
    for s in range(S):
        dup[s, 1 + s] = 1.0
        dup[s, S + 1 + s] = 1.0
    iden = np.eye(128, dtype=np.float32)
    ones1 = np.ones((1, SP), np.float32)
    ones52 = np.ones((S + 1, 1), np.float32)
    initm = np.zeros((SP, BL), np.float32)
    initm[[1, 2, S + 1, S + 2], :] = 1.0
    return {"mc": mc, "dup": dup, "iden": iden, "ones1": ones1,
            "ones52": ones52, "initm": initm}


_CONSTS = _shared_consts()


def _prep_core(predicts, labels, label_lengths, b0):
    """Host-side shard prep for examples [b0, b0+BL)."""
    # permute rows to (t_block, example, t_fine) so streaming tile i = 2j+o
    # holds examples [8o, 8o+8) x timesteps [16j, 16j+16) as 128 contiguous
    # rows (partition p = b_loc*16 + t_fine)
    xs = np.asarray(predicts[b0:b0 + BL], dtype=np.float32)
    xs = np.ascontiguousarray(
        xs.reshape(BL, TBJ, 16, C).transpose(1, 0, 2, 3).reshape(BL * T, C))
    lab = labels[b0:b0 + BL].astype(np.int64)            # [BL, L]
    lens = label_lengths[b0:b0 + BL].astype(np.int64)    # [BL]
    # extended-label class ids per state: even s -> blank 0, odd s -> label
    ext = np.zeros((BL, NI), np.int64)
    ext[:, 1:S:2] = lab
    # ap_gather index tiles: streaming tile i, partition p -> example
    # 8*(i%2) + p//16; slot s holds state-class ext[b][s*16 + p%16]
    i_idx = np.arange(NT)[:, None, None]
    p_idx = np.arange(128)[None, :, None]
    s_idx = np.arange(4)[None, None, :]
    b_of = 8 * (i_idx % 2) + p_idx // 16
    k_of = s_idx * 16 + (p_idx % 16)
    gidx = ext[b_of, k_of]                               # [NT, 128, 4]
    gidx = gidx.transpose(1, 0, 2).reshape(128, NT * 4).astype(np.int16)
    # skip-allowed mask in extended-state space (odd states only, no repeat)
    m51 = np.zeros((BL, S), np.float32)
    m51[:, 3::2] = (lab[:, 1:] != lab[:, :-1]).astype(np.float32)
    # skipm [SP, 16*BL]: row 0 zero, rows 1-51 one, row 52+j col t*16+b =
    # m51[b, j+2]
    skipm = np.zeros((SP, 16, BL), np.float32)
    skipm[1:S + 1] = 1.0
    for jj in range(S - 2):
        skipm[S + 1 + jj, :, :] = m51[:, jj + 2][None, :]
    skipm = skipm.reshape(SP, 16 * BL)
    # selection mask: rows 1+s', cols b
    selt = np.zeros((S + 1, BL), np.float32)
    cols = np.arange(BL)
    selt[2 * lens, cols] = 1.0                           # row 1 + (2len - 1)
    selt[2 * lens + 1, cols] = 1.0                       # row 1 + 2len
    # parity selection for slZ: [128, 2*BL]: cols 0-15 even tiles (e<8),
    # cols 16-31 odd tiles (e>=8)
    bsel = np.zeros((128, 2 * BL), np.float32)
    p = np.arange(128)
    for e in range(8):
        bsel[p // 16 == e, e] = 1.0
    for e in range(8, BL):
        bsel[p // 16 == e - 8, BL + e] = 1.0
    out = {"x": xs, "gidx": gidx, "skipm": skipm, "selt": selt, "bsel": bsel}
    out.update(_CONSTS)
    return out


_NC_CACHE = []


def kernel(predicts, labels, label_lengths):
    predicts = np.asarray(predicts)
    labels = np.asarray(labels)
    label_lengths = np.asarray(label_lengths)
    if not _NC_CACHE:
        _NC_CACHE.append(_build_kernel())
    nc = _NC_CACHE[0]
    in_maps = [
        _prep_core(predicts, labels, label_lengths, k * BL) for k in range(NCORES)
    ]
    res = bass_utils.run_bass_kernel_spmd(nc, in_maps, core_ids=list(range(NCORES)))
    losses = np.concatenate([r["loss16"].reshape(BL) for r in res.results])
    return np.float32(np.mean(losses.astype(np.float64)))
